# revision 8
# baseline (speedup 1.0000x reference)
"""Multi-head attention kernel for 8 Trainium2 NeuronCores.

Strategy: tensor-parallel over heads. Core c owns heads (2c, 2c+1), i.e.
columns [128c, 128c+128) of the projection space.
  - column-parallel Wq/Wk/Wv: each core projects the full token stream onto
    its 128 columns; q/k are produced transposed ([cols, tok]) so the
    attention matmuls contract over the partition dim natively.
  - scores^T = k^T_blk.T @ q^T with softmax along the key dim (= partition),
    normalization deferred: E = exp(scale*scores + mask_bias), U^T = v.T @ E
    with an appended ones row giving sum(E) for free; ctx^T = U^T * (1/sum).
  - row-parallel Wo: each core emits a partial [4096, 1024] output; the host
    sums the 8 partials and adds bo.

Inputs are pre-transposed on the host (query/key/value -> [1024, 4096]) so
all device-side DMA is contiguous.
"""

import numpy as np

import concourse.bass as bass
import concourse.tile as tile
from concourse import bacc, mybir
from concourse.bass_utils import run_bass_kernel_spmd

B, S, D, H = 2, 2048, 1024, 16
DH = D // H          # 64
NCORES = 8
HPC = H // NCORES    # heads per core = 2
CW = HPC * DH        # column width per core = 128
T = B * S            # 4096 tokens
SCALE = 1.0 / np.sqrt(DH)

F32 = mybir.dt.float32
F32R = mybir.dt.float32r

# v_s block layout: per 128-token block: [v_h0 (64) | ones | v_h1 (64) | ones]
VBLK = 2 * (DH + 1)  # 130


def build_nc():
    nc = bacc.Bacc("TRN2", target_bir_lowering=False, debug=False,
                   num_devices=NCORES)

    qT_d = nc.declare_dram_parameter("qT", [D, T], F32R, isOutput=False)
    kT_d = nc.declare_dram_parameter("kT", [D, T], F32R, isOutput=False)
    vT_d = nc.declare_dram_parameter("vT", [D, T], F32R, isOutput=False)
    wq_d = nc.declare_dram_parameter("wq", [D, CW], F32R, isOutput=False)
    wk_d = nc.declare_dram_parameter("wk", [D, CW], F32R, isOutput=False)
    wv_d = nc.declare_dram_parameter("wv", [D, CW], F32R, isOutput=False)
    wo_d = nc.declare_dram_parameter("wo", [CW, D], F32R, isOutput=False)
    bqkv_d = nc.declare_dram_parameter("bqkv", [CW, 3], F32, isOutput=False)
    maskb_d = nc.declare_dram_parameter("maskb", [128, B * (S // 128)], F32,
                                        isOutput=False)
    ident_d = nc.declare_dram_parameter("ident", [128, 128], F32R,
                                        isOutput=False)
    out_d = nc.declare_dram_parameter("out", [T, D], F32, isOutput=True)

    NKT = D // 128       # 8 contraction tiles for projections
    NQC = S // 512       # 4 q-chunks per batch
    NKB = S // 128       # 16 key blocks per batch
    NTB = S // 128       # 16 token blocks per batch

    with tile.TileContext(nc) as tc:
        with (
            tc.tile_pool(name="weights", bufs=1) as wpool,
            tc.tile_pool(name="resident", bufs=1) as rpool,
            tc.tile_pool(name="proj_in", bufs=3) as inpool,
            tc.tile_pool(name="vt_tmp", bufs=2) as vtpool,
            tc.tile_pool(name="E", bufs=6) as epool,
            tc.tile_pool(name="r", bufs=4) as recpool,
            tc.tile_pool(name="Rsb", bufs=2) as rsbpool,
            tc.tile_pool(name="outsb", bufs=4) as outpool,
            tc.tile_pool(name="ps", bufs=8, space="PSUM") as pspool,
        ):
            # ---- load weights / constants (SBUF-resident) ----
            # w*_s[p, kt*CW + m] = w[kt*128 + p, m]
            wq_s = wpool.tile([128, NKT * CW], F32R, tag="wq")
            wk_s = wpool.tile([128, NKT * CW], F32R, tag="wk")
            wv_s = wpool.tile([128, NKT * CW], F32R, tag="wv")
            for w_s, w_d in ((wq_s, wq_d), (wk_s, wk_d), (wv_s, wv_d)):
                nc.sync.dma_start(
                    w_s[:].rearrange("p (kt m) -> p kt m", m=CW),
                    w_d[:, :].rearrange("(kt p) m -> p kt m", p=128))
            wo_s = wpool.tile([128, D], F32R, tag="wo")
            nc.sync.dma_start(wo_s[:], wo_d[:, :])
            bqkv_s = wpool.tile([CW, 3], F32, tag="bqkv")
            nc.sync.dma_start(bqkv_s[:], bqkv_d[:, :])
            maskb_s = wpool.tile([128, B * NKB], F32, tag="maskb")
            nc.sync.dma_start(maskb_s[:], maskb_d[:, :])
            ident_s = wpool.tile([128, 128], F32R, tag="ident")
            nc.sync.dma_start(ident_s[:], ident_d[:, :])
            # rank-1 row masks for the normalizer broadcast:
            # rmask[0, 0:128] selects rows 0..63, rmask[0, 128:256] rows 64..127
            rmask_s = wpool.tile([1, 256], F32R, tag="rmask")
            nc.vector.memset(rmask_s[:, 0:64].bitcast(F32), 1.0)
            nc.vector.memset(rmask_s[:, 64:192].bitcast(F32), 0.0)
            nc.vector.memset(rmask_s[:, 192:256].bitcast(F32), 1.0)

            # ---- per-batch resident activation tiles ----
            qT_s = [rpool.tile([128, S], F32R, tag=f"qT{b}", name=f"qT_s{b}") for b in range(B)]
            kT_s = [rpool.tile([128, S], F32R, tag=f"kT{b}", name=f"kT_s{b}") for b in range(B)]
            v_s = [rpool.tile([128, NTB * VBLK], F32R, tag=f"v{b}", name=f"v_s{b}")
                   for b in range(B)]
            ctxT_s = [rpool.tile([128, S], F32R, tag=f"ctxT{b}", name=f"ctxT_s{b}")
                      for b in range(B)]

            for b in range(B):
                # ones columns interleaved into the v layout
                nc.vector.memset(
                    v_s[b][:].rearrange("p (k j) -> p k j", j=DH + 1)
                    [:, :, DH].bitcast(F32), 1.0)

            # ---- projections, per batch: k, v, then q ----
            def project(dst_evac, w_s, src_d, b):
                """dst_evac(chunk, psum_ap) consumes each [128, 512] result."""
                ps = []
                for kt in range(NKT):
                    src_t = inpool.tile([128, S], F32R, tag="proj_in")
                    nc.sync.dma_start(
                        src_t[:],
                        src_d[kt * 128:(kt + 1) * 128, b * S:(b + 1) * S])
                    for ch in range(NQC):
                        if kt == 0:
                            ps.append(pspool.tile([128, 512], F32, tag="ps", name=f"ps_proj{kt}_{ch}"))
                        nc.tensor.matmul(
                            ps[ch][:],
                            w_s[:, kt * CW:(kt + 1) * CW],
                            src_t[:, ch * 512:(ch + 1) * 512],
                            start=(kt == 0), stop=(kt == NKT - 1))
                for ch in range(NQC):
                    dst_evac(ch, ps[ch])

            for b in range(B):
                # K^T -> kT_s[b]  (bias bk per partition)
                def evac_k(ch, ps, b=b):
                    nc.scalar.activation(
                        kT_s[b][:, ch * 512:(ch + 1) * 512], ps[:],
                        mybir.ActivationFunctionType.Identity,
                        bias=bqkv_s[:, 1:2], scale=1.0)
                project(evac_k, wk_s, kT_d, b)

                # V^T -> vt_tmp, then PE-transpose into v_s[b]
                vt_tmp = vtpool.tile([128, S], F32R, tag="vt_tmp")

                def evac_v(ch, ps, vt_tmp=vt_tmp):
                    nc.scalar.activation(
                        vt_tmp[:, ch * 512:(ch + 1) * 512], ps[:],
                        mybir.ActivationFunctionType.Identity,
                        bias=bqkv_s[:, 2:3], scale=1.0)
                project(evac_v, wv_s, vT_d, b)
                for t in range(NTB):
                    pst = pspool.tile([128, 128], F32R, tag="ps")
                    nc.tensor.transpose(
                        pst[:], vt_tmp[:, t * 128:(t + 1) * 128], ident_s[:])
                    # cols 0:64 = head0, 64:128 = head1; ones col sits between
                    nc.vector.tensor_copy(
                        v_s[b][:, t * VBLK:t * VBLK + DH], pst[:, 0:DH])
                    nc.vector.tensor_copy(
                        v_s[b][:, t * VBLK + DH + 1:t * VBLK + 2 * DH + 1],
                        pst[:, DH:2 * DH])

                # Q^T -> qT_s[b]
                def evac_q(ch, ps, b=b):
                    nc.scalar.activation(
                        qT_s[b][:, ch * 512:(ch + 1) * 512], ps[:],
                        mybir.ActivationFunctionType.Identity,
                        bias=bqkv_s[:, 0:1], scale=1.0)
                project(evac_q, wq_s, qT_d, b)

                # ---- attention for this batch ----
                for qc in range(NQC):
                    qsl = slice(qc * 512, (qc + 1) * 512)
                    psU = [pspool.tile([DH + 1, 512], F32, tag="ps", name=f"psU{b}_{qc}_{h}")
                           for h in range(HPC)]
                    for kb in range(NKB):
                        for h in range(HPC):
                            rows = slice(64 * h, 64 * h + 64)
                            psE = pspool.tile([128, 512], F32, tag="ps")
                            nc.tensor.matmul(
                                psE[:],
                                kT_s[b][rows, kb * 128:(kb + 1) * 128],
                                qT_s[b][rows, qsl],
                                start=True, stop=True)
                            e_sb = epool.tile([128, 512], F32R, tag="E")
                            nc.scalar.activation(
                                e_sb[:], psE[:],
                                mybir.ActivationFunctionType.Exp,
                                bias=maskb_s[:, b * NKB + kb:b * NKB + kb + 1],
                                scale=SCALE)
                            nc.tensor.matmul(
                                psU[h][:],
                                v_s[b][:, kb * VBLK + h * (DH + 1):
                                       kb * VBLK + (h + 1) * (DH + 1)],
                                e_sb[:],
                                start=(kb == 0), stop=(kb == NKB - 1))
                    # normalizers: R[p, n] = 1/sum_h(n) broadcast to rows of h
                    psR = pspool.tile([128, 512], F32, tag="ps")
                    for h in range(HPC):
                        r_h = recpool.tile([1, 512], F32R, tag="r")
                        with nc.allow_low_precision(
                                reason="f32r storage is bit-identical to f32"):
                            nc.vector.reciprocal(r_h[:], psU[h][DH:DH + 1, :])
                        nc.tensor.matmul(
                            psR[:], rmask_s[:, 128 * h:128 * (h + 1)], r_h[:],
                            start=(h == 0), stop=(h == HPC - 1))
                    r_sb = rsbpool.tile([128, 512], F32, tag="Rsb")
                    nc.vector.tensor_copy(r_sb[:], psR[:])
                    for h in range(HPC):
                        rows = slice(64 * h, 64 * h + 64)
                        nc.vector.tensor_mul(
                            ctxT_s[b][rows, qsl], psU[h][0:DH, :],
                            r_sb[rows, :])

                # ---- output projection for this batch ----
                for t in range(NTB):
                    for ch in range(2):
                        psO = pspool.tile([128, 512], F32, tag="ps")
                        nc.tensor.matmul(
                            psO[:],
                            ctxT_s[b][:, t * 128:(t + 1) * 128],
                            wo_s[:, ch * 512:(ch + 1) * 512],
                            start=True, stop=True)
                        o_sb = outpool.tile([128, 512], F32, tag="outsb")
                        nc.vector.tensor_copy(o_sb[:], psO[:])
                        nc.sync.dma_start(
                            out_d[b * S + t * 128:b * S + (t + 1) * 128,
                                  ch * 512:(ch + 1) * 512],
                            o_sb[:])

    nc.compile()
    return nc


_NC_CACHE = []
LAST_RESULT = {}


def kernel(**inputs):
    query = np.ascontiguousarray(np.asarray(inputs["query"], np.float32))
    key = np.ascontiguousarray(np.asarray(inputs["key"], np.float32))
    value = np.ascontiguousarray(np.asarray(inputs["value"], np.float32))
    mask = np.asarray(inputs["mask"], np.float32)
    Wq = np.asarray(inputs["Wq"], np.float32)
    Wk = np.asarray(inputs["Wk"], np.float32)
    Wv = np.asarray(inputs["Wv"], np.float32)
    Wo = np.asarray(inputs["Wo"], np.float32)
    bq = np.asarray(inputs["bq"], np.float32)
    bk = np.asarray(inputs["bk"], np.float32)
    bv = np.asarray(inputs["bv"], np.float32)
    bo = np.asarray(inputs["bo"], np.float32)

    qT = np.ascontiguousarray(query.reshape(T, D).T)
    kT = np.ascontiguousarray(key.reshape(T, D).T)
    vT = np.ascontiguousarray(value.reshape(T, D).T)
    # maskb[p, b*16+kb] = -1e9 * mask[b, 0, 0, kb*128+p]
    maskb = np.ascontiguousarray(
        (mask[:, 0, 0, :] * np.float32(-1e9))
        .reshape(B, S // 128, 128).transpose(2, 0, 1).reshape(128, -1))
    ident = np.eye(128, dtype=np.float32)

    in_maps = []
    for c in range(NCORES):
        cols = slice(CW * c, CW * (c + 1))
        in_maps.append({
            "qT": qT, "kT": kT, "vT": vT,
            "wq": np.ascontiguousarray(Wq[:, cols]),
            "wk": np.ascontiguousarray(Wk[:, cols]),
            "wv": np.ascontiguousarray(Wv[:, cols]),
            "wo": np.ascontiguousarray(Wo[cols, :]),
            "bqkv": np.ascontiguousarray(
                np.stack([bq[cols], bk[cols], bv[cols]], axis=1)),
            "maskb": maskb,
            "ident": ident,
        })

    if not _NC_CACHE:
        _NC_CACHE.append(build_nc())
    nc = _NC_CACHE[0]

    import os
    trace = bool(os.environ.get("KERNEL_TRACE"))
    res = run_bass_kernel_spmd(nc, in_maps, core_ids=list(range(NCORES)),
                               trace=trace)
    LAST_RESULT["res"] = res
    out = np.zeros((T, D), np.float64)
    for c in range(NCORES):
        out += res.results[c]["out"].astype(np.float64)
    out = (out + bo.astype(np.float64)).astype(np.float32)
    return out.reshape(B, S, D)


if __name__ == "__main__":
    import reference
    inputs = {k: np.asarray(v) for k, v in reference.setup_inputs().items()}
    got = kernel(**inputs)
    print("out shape", got.shape, got.dtype)


# revision 9
# speedup vs baseline: 1.5963x; 1.5963x over previous
"""Multi-head attention kernel for 8 Trainium2 NeuronCores.

Strategy: tensor-parallel over heads. Core c owns heads (2c, 2c+1), i.e.
columns [128c, 128c+128) of the projection space.
  - column-parallel Wq/Wk/Wv: each core projects the full token stream onto
    its 128 columns; q/k are produced transposed ([cols, tok]) so the
    attention matmuls contract over the partition dim natively.
  - scores^T = k^T_blk.T @ q^T with softmax along the key dim (= partition),
    normalization deferred: E = exp(scale*scores + mask_bias), U^T = v.T @ E
    with an appended ones row giving sum(E) for free; ctx^T = U^T * (64/sum),
    and the final output is scaled by 1/64 (keeps the reciprocal well inside
    fp16 normal range).
  - row-parallel Wo: each core emits a partial [4096, 1024] output; the host
    sums the 8 partials and adds bo.

Matmul operands are fp16 (PE runs 1 cycle/row and FWL weight loads);
accumulation is fp32 in PSUM. Inputs are pre-transposed and cast to fp16 on
the host so all device-side DMA is contiguous and half-width.
"""

import numpy as np

import concourse.bass as bass
import concourse.tile as tile
from concourse import bacc, mybir
from concourse.bass_utils import run_bass_kernel_spmd

B, S, D, H = 2, 2048, 1024, 16
DH = D // H          # 64
NCORES = 8
HPC = H // NCORES    # heads per core = 2
CW = HPC * DH        # column width per core = 128
T = B * S            # 4096 tokens
SCALE = 1.0 / np.sqrt(DH)
RSCALE = 64.0        # reciprocal pre-scale; undone at output projection

F32 = mybir.dt.float32
F16 = mybir.dt.float16

# v_s block layout: per 128-token block: [v_h0 (64) | ones | v_h1 (64) | ones]
VBLK = 2 * (DH + 1)  # 130

NKT = D // 128       # 8 contraction tiles for projections
NQC = S // 512       # 4 q-chunks per batch
NKB = S // 128       # 16 key blocks per batch
NTB = S // 128       # 16 token blocks per batch


def build_nc():
    nc = bacc.Bacc("TRN2", target_bir_lowering=False, debug=False,
                   num_devices=NCORES)

    qT_d = nc.declare_dram_parameter("qT", [D, T], F16, isOutput=False)
    kT_d = nc.declare_dram_parameter("kT", [D, T], F16, isOutput=False)
    vT_d = nc.declare_dram_parameter("vT", [D, T], F16, isOutput=False)
    wq_d = nc.declare_dram_parameter("wq", [D, CW], F16, isOutput=False)
    wk_d = nc.declare_dram_parameter("wk", [D, CW], F16, isOutput=False)
    wv_d = nc.declare_dram_parameter("wv", [D, CW], F16, isOutput=False)
    wo_d = nc.declare_dram_parameter("wo", [CW, D], F16, isOutput=False)
    bqkv_d = nc.declare_dram_parameter("bqkv", [CW, 3], F32, isOutput=False)
    maskb_d = nc.declare_dram_parameter("maskb", [128, B * NKB], F32,
                                        isOutput=False)
    ident_d = nc.declare_dram_parameter("ident", [128, 128], F16,
                                        isOutput=False)
    out_d = nc.declare_dram_parameter("out", [T, D], F32, isOutput=True)

    with tile.TileContext(nc) as tc:
        with (
            tc.tile_pool(name="weights", bufs=1) as wpool,
            tc.tile_pool(name="resident", bufs=1) as rpool,
            tc.tile_pool(name="proj_in", bufs=4) as inpool,
            tc.tile_pool(name="vt_tmp", bufs=2) as vtpool,
            tc.tile_pool(name="E", bufs=4) as epool,
            tc.tile_pool(name="r", bufs=4) as recpool,
            tc.tile_pool(name="Rsb", bufs=2) as rsbpool,
            tc.tile_pool(name="outsb", bufs=4) as outpool,
            # PSUM: 2x [128,1024] (4 banks) + 2x [65,512] + 2x [128,512]
            tc.tile_pool(name="psA", bufs=2, space="PSUM") as psapool,
            tc.tile_pool(name="psU", bufs=2, space="PSUM") as psupool,
            tc.tile_pool(name="psB", bufs=2, space="PSUM") as psbpool,
        ):
            # ---- load weights / constants (SBUF-resident) ----
            # w*_s[p, kt*CW + m] = w[kt*128 + p, m]
            wq_s = wpool.tile([128, NKT * CW], F16, tag="wq")
            wk_s = wpool.tile([128, NKT * CW], F16, tag="wk")
            wv_s = wpool.tile([128, NKT * CW], F16, tag="wv")
            for w_s, w_d in ((wq_s, wq_d), (wk_s, wk_d), (wv_s, wv_d)):
                nc.sync.dma_start(
                    w_s[:].rearrange("p (kt m) -> p kt m", m=CW),
                    w_d[:, :].rearrange("(kt p) m -> p kt m", p=128))
            wo_s = wpool.tile([128, D], F16, tag="wo")
            nc.sync.dma_start(wo_s[:], wo_d[:, :])
            bqkv_s = wpool.tile([CW, 3], F32, tag="bqkv")
            nc.sync.dma_start(bqkv_s[:], bqkv_d[:, :])
            maskb_s = wpool.tile([128, B * NKB], F32, tag="maskb")
            nc.sync.dma_start(maskb_s[:], maskb_d[:, :])
            ident_s = wpool.tile([128, 128], F16, tag="ident")
            nc.sync.dma_start(ident_s[:], ident_d[:, :])
            # rank-1 row masks for the normalizer broadcast:
            # rmask[0, 0:128] selects rows 0..63, rmask[0, 128:256] rows 64..127
            rmask_s = wpool.tile([1, 256], F16, tag="rmask")
            nc.vector.memset(rmask_s[:, 0:64], 1.0)
            nc.vector.memset(rmask_s[:, 64:192], 0.0)
            nc.vector.memset(rmask_s[:, 192:256], 1.0)

            # ---- per-batch resident activation tiles ----
            qT_s = [rpool.tile([128, S], F16, tag=f"qT{b}", name=f"qT_s{b}")
                    for b in range(B)]
            kT_s = [rpool.tile([128, S], F16, tag=f"kT{b}", name=f"kT_s{b}")
                    for b in range(B)]
            v_s = [rpool.tile([128, NTB * VBLK], F16, tag=f"v{b}",
                              name=f"v_s{b}") for b in range(B)]
            ctxT_s = [rpool.tile([128, S], F16, tag=f"ctxT{b}",
                                 name=f"ctxT_s{b}") for b in range(B)]

            for b in range(B):
                # ones columns interleaved into the v layout
                nc.vector.memset(
                    v_s[b][:].rearrange("p (k j) -> p k j", j=DH + 1)
                    [:, :, DH], 1.0)

            # ---- projections, per batch: k, v, then q ----
            def project(dst_s, bias_col, w_s, src_d, b):
                ps = []
                for kt in range(NKT):
                    src_t = inpool.tile([128, S], F16, tag="proj_in",
                                        name=f"src{b}_{bias_col}_{kt}")
                    nc.sync.dma_start(
                        src_t[:],
                        src_d[kt * 128:(kt + 1) * 128, b * S:(b + 1) * S])
                    for pp in range(NQC // 2):
                        if kt == 0:
                            ps.append(psapool.tile([128, 1024], F32, tag="psA",
                                                   name=f"ps_proj{pp}"))
                        for half in range(2):
                            ch = pp * 2 + half
                            nc.tensor.matmul(
                                ps[pp][:, half * 512:(half + 1) * 512],
                                w_s[:, kt * CW:(kt + 1) * CW],
                                src_t[:, ch * 512:(ch + 1) * 512],
                                start=(kt == 0), stop=(kt == NKT - 1))
                for pp in range(NQC // 2):
                    nc.vector.tensor_scalar_add(
                        dst_s[:, pp * 1024:(pp + 1) * 1024], ps[pp][:],
                        bqkv_s[:, bias_col:bias_col + 1])

            for b in range(B):
                project(kT_s[b], 1, wk_s, kT_d, b)

                # V^T -> vt_tmp, then PE-transpose into v_s[b]
                vt_tmp = vtpool.tile([128, S], F16, tag="vt_tmp",
                                     name=f"vt_tmp{b}")
                project(vt_tmp, 2, wv_s, vT_d, b)
                for t in range(NTB):
                    pst = psbpool.tile([128, 128], F16, tag="psB",
                                       name=f"pst{b}_{t}")
                    nc.tensor.transpose(
                        pst[:], vt_tmp[:, t * 128:(t + 1) * 128], ident_s[:])
                    # cols 0:64 = head0, 64:128 = head1; ones col sits between
                    nc.vector.tensor_copy(
                        v_s[b][:, t * VBLK:t * VBLK + DH], pst[:, 0:DH])
                    nc.vector.tensor_copy(
                        v_s[b][:, t * VBLK + DH + 1:t * VBLK + 2 * DH + 1],
                        pst[:, DH:2 * DH])

                project(qT_s[b], 0, wq_s, qT_d, b)

                # ---- attention for this batch ----
                for qc in range(NQC):
                    qsl = slice(qc * 512, (qc + 1) * 512)
                    psU = [psupool.tile([DH + 1, 512], F32, tag="psU",
                                        name=f"psU{b}_{qc}_{h}")
                           for h in range(HPC)]
                    for kb in range(NKB):
                        psE = psapool.tile([128, 1024], F32, tag="psA",
                                           name=f"psE{b}_{qc}_{kb}")
                        for h in range(HPC):
                            rows = slice(64 * h, 64 * h + 64)
                            nc.tensor.matmul(
                                psE[:, h * 512:(h + 1) * 512],
                                kT_s[b][rows, kb * 128:(kb + 1) * 128],
                                qT_s[b][rows, qsl],
                                start=True, stop=True)
                        e_sb = epool.tile([128, 1024], F16, tag="E",
                                          name=f"e{b}_{qc}_{kb}")
                        nc.scalar.activation(
                            e_sb[:], psE[:],
                            mybir.ActivationFunctionType.Exp,
                            bias=maskb_s[:, b * NKB + kb:b * NKB + kb + 1],
                            scale=SCALE)
                        for h in range(HPC):
                            nc.tensor.matmul(
                                psU[h][:],
                                v_s[b][:, kb * VBLK + h * (DH + 1):
                                       kb * VBLK + (h + 1) * (DH + 1)],
                                e_sb[:, h * 512:(h + 1) * 512],
                                start=(kb == 0), stop=(kb == NKB - 1))
                    # normalizers: R[p, n] = RSCALE/sum_h(n) on rows of head h
                    psR = psbpool.tile([128, 512], F32, tag="psB",
                                       name=f"psR{b}_{qc}")
                    for h in range(HPC):
                        r32 = recpool.tile([1, 512], F32, tag="r",
                                           name=f"r32_{b}_{qc}_{h}")
                        with nc.allow_low_precision(
                                reason="normalizer reciprocal, fp32 out"):
                            nc.vector.reciprocal(r32[:], psU[h][DH:DH + 1, :])
                        r16 = recpool.tile([1, 512], F16, tag="r16",
                                           name=f"r16_{b}_{qc}_{h}")
                        nc.vector.tensor_scalar_mul(r16[:], r32[:], RSCALE)
                        nc.tensor.matmul(
                            psR[:], rmask_s[:, 128 * h:128 * (h + 1)], r16[:],
                            start=(h == 0), stop=(h == HPC - 1))
                    r_sb = rsbpool.tile([128, 512], F32, tag="Rsb",
                                        name=f"r_sb{b}_{qc}")
                    nc.vector.tensor_copy(r_sb[:], psR[:])
                    for h in range(HPC):
                        rows = slice(64 * h, 64 * h + 64)
                        nc.vector.tensor_mul(
                            ctxT_s[b][rows, qsl], psU[h][0:DH, :],
                            r_sb[rows, :])

                # ---- output projection for this batch ----
                for t in range(NTB):
                    psO = psapool.tile([128, 1024], F32, tag="psA",
                                       name=f"psO{b}_{t}")
                    for ch in range(2):
                        nc.tensor.matmul(
                            psO[:, ch * 512:(ch + 1) * 512],
                            ctxT_s[b][:, t * 128:(t + 1) * 128],
                            wo_s[:, ch * 512:(ch + 1) * 512],
                            start=True, stop=True)
                    o_sb = outpool.tile([128, 1024], F32, tag="outsb",
                                        name=f"o_sb{b}_{t}")
                    nc.vector.tensor_scalar_mul(o_sb[:], psO[:],
                                                1.0 / RSCALE)
                    nc.sync.dma_start(
                        out_d[b * S + t * 128:b * S + (t + 1) * 128, :],
                        o_sb[:])

    nc.compile()
    return nc


_NC_CACHE = []
LAST_RESULT = {}


def kernel(**inputs):
    query = np.asarray(inputs["query"], np.float32)
    key = np.asarray(inputs["key"], np.float32)
    value = np.asarray(inputs["value"], np.float32)
    mask = np.asarray(inputs["mask"], np.float32)
    Wq = np.asarray(inputs["Wq"], np.float32)
    Wk = np.asarray(inputs["Wk"], np.float32)
    Wv = np.asarray(inputs["Wv"], np.float32)
    Wo = np.asarray(inputs["Wo"], np.float32)
    bq = np.asarray(inputs["bq"], np.float32)
    bk = np.asarray(inputs["bk"], np.float32)
    bv = np.asarray(inputs["bv"], np.float32)
    bo = np.asarray(inputs["bo"], np.float32)

    qT = np.ascontiguousarray(query.reshape(T, D).T.astype(np.float16))
    kT = np.ascontiguousarray(key.reshape(T, D).T.astype(np.float16))
    vT = np.ascontiguousarray(value.reshape(T, D).T.astype(np.float16))
    # maskb[p, b*16+kb] = -1e9 * mask[b, 0, 0, kb*128+p]
    maskb = np.ascontiguousarray(
        (mask[:, 0, 0, :] * np.float32(-1e9))
        .reshape(B, S // 128, 128).transpose(2, 0, 1).reshape(128, -1))
    ident = np.eye(128, dtype=np.float16)

    in_maps = []
    for c in range(NCORES):
        cols = slice(CW * c, CW * (c + 1))
        in_maps.append({
            "qT": qT, "kT": kT, "vT": vT,
            "wq": np.ascontiguousarray(Wq[:, cols].astype(np.float16)),
            "wk": np.ascontiguousarray(Wk[:, cols].astype(np.float16)),
            "wv": np.ascontiguousarray(Wv[:, cols].astype(np.float16)),
            "wo": np.ascontiguousarray(Wo[cols, :].astype(np.float16)),
            "bqkv": np.ascontiguousarray(
                np.stack([bq[cols], bk[cols], bv[cols]], axis=1)),
            "maskb": maskb,
            "ident": ident,
        })

    if not _NC_CACHE:
        _NC_CACHE.append(build_nc())
    nc = _NC_CACHE[0]

    import os
    trace = bool(os.environ.get("KERNEL_TRACE"))
    res = run_bass_kernel_spmd(nc, in_maps, core_ids=list(range(NCORES)),
                               trace=trace)
    LAST_RESULT["res"] = res
    out = np.zeros((T, D), np.float64)
    for c in range(NCORES):
        out += res.results[c]["out"].astype(np.float64)
    out = (out + bo.astype(np.float64)).astype(np.float32)
    return out.reshape(B, S, D)
